# revision 18
# baseline (speedup 1.0000x reference)
"""ArcFace (AngularPenaltySMLoss) distributed Trainium2 kernel, v8.

Strategy (tensor-parallel over classes, per the sharding hint):
  - Shard W's C=100000 rows over 8 cores (12500 each).
  - Host: normalize x; pre-scale and cast x, W to fp8e4m3; lay both out
    chunk-contiguously so every DMA is 128 straight partition lines.
    All input DMAs ride ONE queue in need-order (x quarter tiles and
    the small first W chunk first) so nothing is starved.
  - Device (SPMD, no collectives): per (chunk, b-tile) unit, fp8
    DoubleRow matmuls fill TWO bank-aligned PSUM tiles: psa (cols
    0:1024) and psd (cols 1024:w). Tile/PSUM semantics serialize all
    consumers of one PSUM tile and make them wait for the whole fill,
    so each tile gets exactly ONE consumer:
      * psa -> ACT: exp(2*raw) + accum_out (free-dim sum straight into
        an accumulator slot; the exp value output goes to a dead SBUF
        dump tile).
      * psd -> DVE: Schraudolph bit-trick exp — tensor_scalar affine
        fp32->int16 (bits of bf16 exp), then one scalar_tensor_tensor
        fold-add over the bitcast-bf16 halves with accum_out. Every
        7th unit ACT takes psd instead (exp+accum) to balance engine
        load (~95% each, under the PE fill rate).
    ACT and DVE accumulate into SEPARATE tiles — a shared tile would
    serialize the two engines through Tile's write-order tracking.
  - Final per-bt reduce of each accumulator + [128, 24] DMA out; host
    adds the halves.
  - Host: sum partials over cores, compute the tiny per-sample target /
    arccos / log path in f64, return the scalar loss.
"""

import sys

if "/opt/trn_rl_repo" not in sys.path:
    sys.path.insert(0, "/opt/trn_rl_repo")

import ml_dtypes
import numpy as np

import concourse.bass as bass
import concourse.mybir as mybir
from concourse import bacc
from concourse.bass_utils import run_bass_kernel_spmd
from concourse.tile import TileContext

B, C, D = 1024, 100000, 512
S_SCALE, MARGIN, EPS = 64.0, 0.5, 1e-7
N_CORES = 8
C_SHARD = C // N_CORES          # 12500
P = 128
KO = D // P                     # 4 k-chunks of 128
B_TILES = B // P                # 8
HB = B // 2                     # x tile half-batch (512)
MM_N = 512                      # one matmul output <= one PSUM bank
HALF = 1024                     # psa width (2 PSUM banks)
N_WARM = 4                      # PE warm-up matmuls (bridge DMA fill + HAM)

WSCALE, XSCALE = 8.0, 4.0       # fp8 pre-scales (folded out via ACT_SCALE)
ACT_SCALE = S_SCALE / (WSCALE * XSCALE)   # 2.0

# Schraudolph bf16 exp bits: i16 = rint(A * raw + Badd); bitcast bf16.
SCH_A = ACT_SCALE * 184.66496580927726
SCH_B = 16256.0 - 7.4

CHUNKS = [512, 1748, 2048, 2048, 2048, 2048, 2048]
assert sum(CHUNKS) == C_SHARD
N_CHUNKS = len(CHUNKS)
ACT_PSD_EVERY = 7               # every 7th unit ACT consumes psd too

LAST_RESULT = None
_NC_CACHE = None


def _build_bass():
    nc = bacc.Bacc("TRN2")
    xnt = nc.declare_dram_parameter("xnt", [P, KO * B], mybir.dt.float8e4, isOutput=False)
    wt = nc.declare_dram_parameter("wt", [P, KO * C_SHARD], mybir.dt.float8e4, isOutput=False)
    out_a = nc.declare_dram_parameter(
        "out_a", [P, B_TILES * 2 * N_CHUNKS], mybir.dt.float32, isOutput=True
    )
    out_d = nc.declare_dram_parameter(
        "out_d", [P, B_TILES * N_CHUNKS], mybir.dt.float32, isOutput=True
    )

    fp8 = mybir.dt.float8e4
    f32 = mybir.dt.float32
    bf16 = mybir.dt.bfloat16
    i16 = mybir.dt.int16
    DR = mybir.MatmulPerfMode.DoubleRow
    EXP = mybir.ActivationFunctionType.Exp

    with TileContext(nc) as tc:
        with (
            tc.tile_pool(name="xp", bufs=1) as xp,
            tc.tile_pool(name="wp", bufs=1) as wp,
            tc.tile_pool(name="ip", bufs=3) as ip,
            tc.tile_pool(name="ep", bufs=3) as ep,
            tc.tile_pool(name="fp", bufs=2) as fpool,
            tc.tile_pool(name="ac", bufs=1) as ac,
            # one PSUM pool per consumer engine; single reader per tile
            tc.tile_pool(name="psa", bufs=2, space="PSUM") as psa_p,
            tc.tile_pool(name="psd", bufs=2, space="PSUM") as psd_p,
        ):
            # x quarter tiles (k-half x batch-half) + W chunks, all on one
            # queue ordered by first use. dram xnt layout: [p, ko, b].
            xt = {}
            for kh in (0, 1):
                for bh in (0, 1):
                    xt[(kh, bh)] = xp.tile(
                        [P, 2, HB], fp8, tag=f"x{kh}{bh}", name=f"x{kh}{bh}"
                    )

            def dma_x(kh, bh):
                src = xnt.rearrange("p (ko b) -> p ko b", ko=KO)[
                    :, 2 * kh : 2 * kh + 2, bh * HB : (bh + 1) * HB
                ]
                nc.sync.dma_start(xt[(kh, bh)][:], src)

            wts = []
            for ci, cw in enumerate(CHUNKS):
                wts.append(
                    wp.tile([P, KO, cw], fp8, tag=f"wt{ci}", name=f"wt{ci}")
                )

            def dma_w(ci):
                c0 = sum(CHUNKS[:ci])
                cw = CHUNKS[ci]
                nc.sync.dma_start(wts[ci][:], wt[:, 4 * c0 : 4 * (c0 + cw)])

            dma_x(0, 0)
            dma_w(0)
            dma_x(1, 0)
            dma_x(0, 1)
            dma_x(1, 1)
            for ci in range(1, N_CHUNKS):
                dma_w(ci)

            # ACT table warm-up: a tiny exp before any real work so the
            # ~2.7us PSEUDO_LOAD_ACT_FUNC_SET runs during the DMA fill.
            jt = xp.tile([P, 8], f32)
            nc.vector.memset(jt[:], 0.0)
            ja = xp.tile([P, 8], bf16)
            nc.scalar.activation(ja[:], jt[:], EXP)

            # PE warm-up: bridge from engine start to the first
            # data-dependent matmul so HAM un-throttles (~3.4us window).
            wsrc = xp.tile([P, MM_N], fp8, tag="warm_src")
            nc.vector.memset(wsrc[:], 1)
            for wi in range(N_WARM):
                pw = (psa_p if wi % 2 == 0 else psd_p).tile(
                    [P, HALF], f32, tag="ps", name="pw"
                )
                nc.tensor.matmul(
                    pw[:, :MM_N], wsrc[:, :P], wsrc[:], start=True, stop=True
                )

            # separate accumulators per engine; acc_a has 2 slots/unit
            # (the every-7th psd exp), zeroed once.
            acc_a = ac.tile([P, B_TILES, 2 * N_CHUNKS], f32)
            acc_d = ac.tile([P, B_TILES, N_CHUNKS], f32)
            nc.vector.memset(acc_a[:], 0.0)
            nc.vector.memset(acc_d[:], 0.0)

            ui = 0
            for ci, cw in enumerate(CHUNKS):
                wt_t = wts[ci]
                wa = min(cw, HALF)          # psa columns
                wd = cw - wa                # psd columns
                for bt in range(B_TILES):
                    n_bridge = 5 if (ci == 0 and bt == 4) else (
                        2 if (ci == 1 and bt == 0) else 0
                    )
                    if n_bridge:
                        # bridge matmuls: keep the PE (and HAM's activity
                        # window) busy while the next DMA lands.
                        for wi in range(n_bridge):
                            pw2 = (psa_p if wi % 2 == 0 else psd_p).tile(
                                [P, HALF], f32, tag="ps", name="pw2"
                            )
                            nc.tensor.matmul(
                                pw2[:, :MM_N], wsrc[:, :P], wsrc[:],
                                start=True, stop=True,
                            )
                    bh, bo = divmod(bt, 4)

                    def lhs(k):
                        return xt[(k, bh)][:, :, bo * P : (bo + 1) * P]

                    pa = psa_p.tile([P, HALF], f32, tag="ps", name="pa")
                    for si in range((wa + MM_N - 1) // MM_N):
                        s0 = si * MM_N
                        sw = min(MM_N, wa - s0)
                        for k in (0, 1):
                            nc.tensor.matmul(
                                pa[:, s0 : s0 + sw],
                                lhs(k),
                                wt_t[:, 2 * k : 2 * k + 2, s0 : s0 + sw],
                                start=(k == 0),
                                stop=(k == 1),
                                perf_mode=DR,
                            )
                    if wd:
                        pd = psd_p.tile([P, HALF], f32, tag="ps", name="pd")
                        for si in range((wd + MM_N - 1) // MM_N):
                            s0 = si * MM_N
                            sw = min(MM_N, wd - s0)
                            for k in (0, 1):
                                nc.tensor.matmul(
                                    pd[:, s0 : s0 + sw],
                                    lhs(k),
                                    wt_t[:, 2 * k : 2 * k + 2, wa + s0 : wa + s0 + sw],
                                    start=(k == 0),
                                    stop=(k == 1),
                                    perf_mode=DR,
                                )

                    # psa -> ACT exp + accumulate (value output is dead)
                    exd = ep.tile([P, HALF], bf16, tag="exd")
                    nc.scalar.activation(
                        exd[:, :wa],
                        pa[:, :wa],
                        EXP,
                        scale=ACT_SCALE,
                        accum_out=acc_a[:, bt, 2 * ci : 2 * ci + 1],
                    )
                    if wd:
                        if wd == HALF and ui % ACT_PSD_EVERY == ACT_PSD_EVERY - 1:
                            # balance: ACT takes psd on this unit
                            exd2 = ep.tile([P, HALF], bf16, tag="exd")
                            nc.scalar.activation(
                                exd2[:, :wd],
                                pd[:, :wd],
                                EXP,
                                scale=ACT_SCALE,
                                accum_out=acc_a[:, bt, 2 * ci + 1 : 2 * ci + 2],
                            )
                        else:
                            # psd -> DVE Schraudolph + fused fold+accum
                            h = wd // 2
                            it = ip.tile([P, HALF], i16, tag="it")
                            nc.vector.tensor_scalar(
                                it[:, :wd],
                                pd[:, :wd],
                                SCH_A,
                                SCH_B,
                                mybir.AluOpType.mult,
                                mybir.AluOpType.add,
                            )
                            fo = fpool.tile([P, 512], bf16, tag="fo")
                            nc.vector.scalar_tensor_tensor(
                                fo[:, :h],
                                it[:, 0:h].bitcast(bf16),
                                1.0,
                                it[:, h:wd].bitcast(bf16),
                                mybir.AluOpType.mult,
                                mybir.AluOpType.add,
                                accum_out=acc_d[:, bt, ci : ci + 1],
                            )
                    ui += 1

            # ship raw accumulator slots; the host does the tiny final sum
            nc.scalar.dma_start(out_a[:], acc_a[:])
            nc.sync.dma_start(out_d[:], acc_d[:])

    nc.compile()
    return nc


def _get_nc():
    global _NC_CACHE
    if _NC_CACHE is None:
        _NC_CACHE = _build_bass()
    return _NC_CACHE


def kernel(x: np.ndarray, labels: np.ndarray, W: np.ndarray) -> np.ndarray:
    global LAST_RESULT
    x = np.asarray(x, dtype=np.float32)
    W = np.asarray(W, dtype=np.float32)
    labels = np.asarray(labels)

    # ---- host prep (sharding glue) ----
    norms = np.maximum(np.sqrt((x.astype(np.float64) ** 2).sum(axis=1)), 1e-12)
    xn = (x / norms[:, None].astype(np.float32)).astype(np.float32)
    # xnt[p, ko, b] = xn[b, ko*128+p] * XSCALE
    xq = (
        np.ascontiguousarray(
            (xn.T * XSCALE).reshape(KO, P, B).transpose(1, 0, 2)
        )
        .astype(ml_dtypes.float8_e4m3)
        .reshape(P, KO * B)
    )

    in_maps = []
    for i in range(N_CORES):
        shard = W[i * C_SHARD : (i + 1) * C_SHARD]
        blocks = []
        c0 = 0
        for cw in CHUNKS:
            blk = (shard[c0 : c0 + cw].T * WSCALE).reshape(KO, P, cw)
            blocks.append(blk.transpose(1, 0, 2).reshape(P, KO * cw))
            c0 += cw
        wt_q = np.concatenate(blocks, axis=1).astype(ml_dtypes.float8_e4m3)
        in_maps.append({"xnt": xq, "wt": np.ascontiguousarray(wt_q)})

    # ---- device: per-core partial sum over classes of exp(s*logit) ----
    nc = _get_nc()
    res = run_bass_kernel_spmd(nc, in_maps, core_ids=list(range(N_CORES)))
    LAST_RESULT = res

    # ---- host combine (the all-reduce + tiny per-sample tail) ----
    sumexp = np.zeros(B, dtype=np.float64)
    for i in range(N_CORES):
        pa = res.results[i]["out_a"].astype(np.float64)
        pd = res.results[i]["out_d"].astype(np.float64)
        part = (
            pa.reshape(P, B_TILES, 2 * N_CHUNKS).sum(axis=2)
            + pd.reshape(P, B_TILES, N_CHUNKS).sum(axis=2)
        )                                                # [P, B_TILES]
        sumexp += part.T.reshape(B)                      # b = bt*128 + p

    target = np.einsum(
        "bd,bd->b", xn.astype(np.float64), W[labels].astype(np.float64)
    )
    tgt = np.clip(target, -1.0 + EPS, 1.0 - EPS)
    numerator = S_SCALE * np.cos(np.arccos(tgt) + MARGIN)
    excl = sumexp - np.exp(S_SCALE * tgt)
    L = numerator - np.log(np.exp(numerator) + excl)
    return np.array(-L.mean(), dtype=np.float32)
